# revision 1
# baseline (speedup 1.0000x reference)
"""FM bi-interaction (embedding_lookup) Trainium2 kernel.

out[n, k] = 0.5 * ((x @ E)^2 - (x*x) @ (E*E))[n, k] * mask[n]
mask[n] = 1 if n in train_idx else 0

Sharding: data-parallel over the 20000 input rows, 2500 rows per core on 8
NeuronCores; the [10000, 32] embedding table is replicated. Per core, x is
uploaded in f-major (transposed) layout so the contraction dim lands on SBUF
partitions with fully contiguous 1 MB DMAs. The train_idx mask is built
on-device without dynamic DMA (disabled in this runtime): each core receives
the indices that fall in its row range (rebased, padded with sentinel ROWS);
per 128-index batch, DVE computes eq[p, n] = (iota[n] == idx[p]) and an
all-ones [128, 32] matmul reduces eq over partitions into match counts,
replicated across the 32 output partitions; min(count, 1) * 0.5 gives the
half-mask applied in the epilogue.

Matmuls run in float32r (full-rate fp32 on the PE, ~1.5e-4 relative
accuracy), accumulating 80 f-tiles of 125 contraction rows into PSUM per
500-column output chunk.
"""

import math
import sys

if "/opt/trn_rl_repo" not in sys.path:
    sys.path.insert(0, "/opt/trn_rl_repo")

import numpy as np

N_ROWS = 20000
F = 10000
EK = 32
CORES = 8
ROWS = N_ROWS // CORES  # 2500 rows per core
NCHUNK = 500
CHUNKS = ROWS // NCHUNK  # 5
FP = 125  # contraction rows per f-tile (125 * 80 = 10000)
FTILES = F // FP  # 80

_PROGRAM_CACHE: dict = {}


def _build_program(k_idx: int):
    """Per-core Bass program. k_idx = number of 128-index scatter batches."""
    import concourse.bass as bass
    import concourse.mybir as mybir
    import concourse.tile as tile
    from concourse import bacc

    f32 = mybir.dt.float32
    f32r = mybir.dt.float32r
    bf16 = mybir.dt.bfloat16

    nc = bacc.Bacc("TRN2", target_bir_lowering=False, debug=False)
    xt = nc.dram_tensor("xt", [F, ROWS], f32r, kind="ExternalInput")
    emb = nc.dram_tensor("emb", [F, EK], f32r, kind="ExternalInput")
    # train indices as floats, padded with ROWS (matches nothing in iota)
    idxf = nc.dram_tensor("idxf", [128, k_idx], f32, kind="ExternalInput")
    iota_in = nc.dram_tensor("iota", [128, ROWS], f32, kind="ExternalInput")
    outT = nc.dram_tensor("outT", [EK, ROWS], f32, kind="ExternalOutput")

    with tile.TileContext(nc) as tc:
        with (
            tc.tile_pool(name="wpool", bufs=1) as wpool,
            tc.tile_pool(name="mpool", bufs=1) as mpool,
            tc.tile_pool(name="xpool", bufs=10) as xpool,
            tc.tile_pool(name="qpool", bufs=4) as qpool,
            tc.tile_pool(name="opool", bufs=2) as opool,
        ):
            # Embedding table (and its elementwise square) as stationary
            # operands: 80 f-tiles of [125, 32] each.
            e_sb = wpool.tile([FP, FTILES, EK], f32r)
            nc.sync.dma_start(
                out=e_sb[:], in_=emb[:].rearrange("(a p) k -> p a k", p=FP)
            )
            e2_sb = wpool.tile([FP, FTILES, EK], f32r)
            nc.vector.tensor_mul(e2_sb[:], e_sb[:], e_sb[:])

            # Half-mask (values 0 / 0.5), built without dynamic DMA:
            # eq[p, n] = (n == idx[p, j]) in bf16 (0/1 exact); an all-ones
            # bf16 [128, 32] matmul sums eq over partitions (match count) and
            # replicates the row across 32 output partitions. Counts for
            # chunks 0-3 are packed into ONE persistent PSUM bank at
            # partition offsets 32c (col-tiling); chunk 4 uses a second bank.
            # min(count, 1) * 0.5 is fused into the epilogue read.
            iota_sb = mpool.tile([128, ROWS], f32)
            nc.sync.dma_start(out=iota_sb[:], in_=iota_in[:])
            idx_sb = mpool.tile([128, k_idx], f32)
            nc.sync.dma_start(out=idx_sb[:], in_=idxf[:])
            ones_sb = mpool.tile([128, EK], bf16)
            nc.gpsimd.memset(ones_sb[:], 1.0)

            QUAD = 4
            MSLOTS = FTILES // QUAD  # mask batches that fit in chunk 0
            ps_ctx = tc.tile_pool(name="pspool", bufs=3, space="PSUM")
            pspool = ps_ctx.__enter__()
            eq_ctx = tc.tile_pool(name="eqpool", bufs=2)
            eqpool = eq_ctx.__enter__()
            psMaskA = pspool.tile([128, 512], f32, space="PSUM", bufs=1)
            psMaskB = pspool.tile([EK, 512], f32, space="PSUM", bufs=1)

            def emit_mask_batch(j):
                eq = eqpool.tile([128, ROWS], bf16, name="eq")
                nc.vector.tensor_tensor(
                    out=eq[:],
                    in0=iota_sb[:],
                    in1=idx_sb[:, j : j + 1].broadcast_to([128, ROWS]),
                    op=mybir.AluOpType.is_equal,
                )
                for cc in range(CHUNKS):
                    tgt = (
                        psMaskA[32 * cc : 32 * cc + 32, :NCHUNK]
                        if cc < 4
                        else psMaskB[:, :NCHUNK]
                    )
                    nc.tensor.matmul(
                        tgt,
                        ones_sb[:],
                        eq[:, cc * NCHUNK : (cc + 1) * NCHUNK],
                        start=(j == 0),
                        stop=(j == k_idx - 1),
                        tile_position=(0, 32 * cc if cc < 4 else 0),
                    )

            interleaved = k_idx <= MSLOTS
            if not interleaved:
                for j in range(k_idx):
                    emit_mask_batch(j)

            for c in range(CHUNKS):
                ns = slice(c * NCHUNK, (c + 1) * NCHUNK)
                psL = pspool.tile([EK, NCHUNK], f32, space="PSUM")
                psR = pspool.tile([EK, NCHUNK], f32, space="PSUM")
                for q in range(FTILES // QUAD):
                    t0 = q * QUAD
                    xt_sb = xpool.tile([FP, QUAD, NCHUNK], f32r)
                    nc.sync.dma_start(
                        out=xt_sb[:],
                        in_=xt[t0 * FP : (t0 + QUAD) * FP, ns].rearrange(
                            "(a p) n -> p a n", p=FP
                        ),
                    )
                    xq_sb = qpool.tile([FP, QUAD, NCHUNK], f32r)
                    nc.vector.tensor_mul(xq_sb[:], xt_sb[:], xt_sb[:])
                    for h in range(QUAD):
                        t = t0 + h
                        nc.tensor.matmul(
                            psL[:],
                            e_sb[:, t, :],
                            xt_sb[:, h, :],
                            start=(t == 0),
                            stop=(t == FTILES - 1),
                        )
                        nc.tensor.matmul(
                            psR[:],
                            e2_sb[:, t, :],
                            xq_sb[:, h, :],
                            start=(t == 0),
                            stop=(t == FTILES - 1),
                        )
                    if interleaved and c == 0 and q < k_idx:
                        emit_mask_batch(q)
                # out = (L*L - R) * (min(count, 1) * 0.5)
                msrc = (
                    psMaskA[32 * c : 32 * c + 32, :NCHUNK]
                    if c < 4
                    else psMaskB[:, :NCHUNK]
                )
                msb = opool.tile([EK, NCHUNK], f32)
                nc.vector.tensor_scalar(
                    out=msb[:],
                    in0=msrc,
                    scalar1=1.0,
                    scalar2=0.5,
                    op0=mybir.AluOpType.min,
                    op1=mybir.AluOpType.mult,
                )
                lsb = opool.tile([EK, NCHUNK], f32)
                nc.vector.tensor_copy(lsb[:], psL[:])
                osb = opool.tile([EK, NCHUNK], f32)
                nc.vector.tensor_mul(osb[:], lsb[:], lsb[:])
                nc.vector.tensor_sub(osb[:], osb[:], psR[:])
                nc.vector.tensor_mul(osb[:], osb[:], msb[:])
                nc.sync.dma_start(out=outT[:, ns], in_=osb[:])
            eq_ctx.__exit__(None, None, None)
            ps_ctx.__exit__(None, None, None)

    nc.compile()
    return nc


def _get_program(k_idx: int):
    if k_idx not in _PROGRAM_CACHE:
        _PROGRAM_CACHE[k_idx] = _build_program(k_idx)
    return _PROGRAM_CACHE[k_idx]


def _prepare_in_maps(input, emb_weight, train_idx):
    x = np.asarray(input, dtype=np.float32)
    e = np.ascontiguousarray(np.asarray(emb_weight, dtype=np.float32))
    idx = np.asarray(train_idx).astype(np.int64)

    per_core_idx = []
    max_cnt = 1
    for c in range(CORES):
        lo = c * ROWS
        sel = idx[(idx >= lo) & (idx < lo + ROWS)] - lo
        sel = sel.astype(np.int32)
        per_core_idx.append(sel)
        max_cnt = max(max_cnt, len(sel))
    k_idx = max(1, math.ceil(max_cnt / 128))
    pad_len = 128 * k_idx

    iota = np.ascontiguousarray(
        np.broadcast_to(np.arange(ROWS, dtype=np.float32), (128, ROWS))
    )
    in_maps = []
    for c in range(CORES):
        sel = per_core_idx[c]
        padded = np.full(pad_len, ROWS, dtype=np.float32)  # ROWS matches nothing
        padded[: len(sel)] = sel.astype(np.float32)
        xt = np.ascontiguousarray(x[c * ROWS : (c + 1) * ROWS, :].T)
        in_maps.append(
            {
                "xt": xt,
                "emb": e,
                "idxf": padded.reshape(128, k_idx),
                "iota": iota,
            }
        )
    return in_maps, k_idx


def run_sharded(input, emb_weight, train_idx, trace: bool = False):
    """Run on 8 cores; returns (full_output, BassKernelResults)."""
    from concourse.bass_utils import run_bass_kernel_spmd

    in_maps, k_idx = _prepare_in_maps(input, emb_weight, train_idx)
    nc = _get_program(k_idx)
    res = run_bass_kernel_spmd(
        nc, in_maps, core_ids=list(range(CORES)), trace=trace
    )
    out = np.empty((N_ROWS, EK), dtype=np.float32)
    for c in range(CORES):
        out[c * ROWS : (c + 1) * ROWS, :] = res.results[c]["outT"].T
    return out, res


def kernel(input, emb_weight, train_idx):
    out, _ = run_sharded(input, emb_weight, train_idx)
    return out



# revision 2
# speedup vs baseline: 7.1998x; 7.1998x over previous
"""FM bi-interaction (embedding_lookup) Trainium2 kernel.

out[n, k] = 0.5 * ((x @ E)^2 - (x*x) @ (E*E))[n, k] * mask[n]
mask[n] = 1 if n in train_idx else 0

Strategy (data-parallel over rows, 8 NeuronCores):
- Only rows present in train_idx have nonzero output (~11k of 20k). The host
  gathers the unique train rows, splits them evenly across the 8 cores, and
  scatters the per-row results back into a zero output — no on-device mask.
- x is uploaded in bf16 (the 2e-2 rel-err gate leaves ~40x headroom), halving
  HBM traffic; E is pre-scaled by 1/sqrt(2) on the host so the 0.5 factor
  folds into the matmuls (out = L^2 - R with L = x@E', R = x^2@E'^2).
- Host packs x into the exact SBUF tile layout ([128 f-partitions, 8 f-tiles,
  w rows] per block, f padded 10000->10240), so every x DMA is one ~0.9 MB
  transfer with fully contiguous 7.3 KB per-partition lines.
- Matmuls are M=32 (EK) wide; four accumulation streams (L-even/L-odd tiles,
  R-even/R-odd tiles) run in distinct 32-column PE groups via tile_position,
  accumulating into one PSUM bank [128, w]. The epilogue folds the partition
  groups with DVE adds and computes L*L - R, then DMAs [32, w] out.
"""

import math
import sys

if "/opt/trn_rl_repo" not in sys.path:
    sys.path.insert(0, "/opt/trn_rl_repo")

import numpy as np

N_ROWS = 20000
F = 10000
EK = 32
CORES = 8
FP = 128  # f-rows per tile (on SBUF partitions)
FTILES = 80
F_PAD = FP * FTILES  # 10240
OCT = 8  # f-tiles per DMA block
NOCT = FTILES // OCT  # 10
MAXW = 512  # PSUM bank limit (f32 columns)

_PROGRAM_CACHE: dict = {}


def _build_program(nch: int, w: int):
    """Per-core Bass program: nch chunks of w rows each (w <= 512)."""
    import concourse.mybir as mybir
    import concourse.tile as tile
    from concourse import bacc

    f32 = mybir.dt.float32
    bf16 = mybir.dt.bfloat16

    P = nch * w
    nc = bacc.Bacc("TRN2", target_bir_lowering=False, debug=False)
    # packed x: per partition p, flat index (c*FTILES + t)*w + j holds
    # x[row base_c + j, f = t*128 + p] (bf16, f >= 10000 zero-padded)
    xt = nc.dram_tensor("xt", [128, FTILES * P], bf16, kind="ExternalInput")
    # packed E/sqrt(2): per partition p, flat t*EK + k = E'[t*128 + p, k]
    emb = nc.dram_tensor("emb", [128, FTILES * EK], bf16, kind="ExternalInput")
    outT = nc.dram_tensor("outT", [EK, P], f32, kind="ExternalOutput")

    with tile.TileContext(nc) as tc:
        with (
            tc.tile_pool(name="wpool", bufs=1) as wpool,
            tc.tile_pool(name="xpool", bufs=4) as xpool,
            tc.tile_pool(name="qpool", bufs=4) as qpool,
            tc.tile_pool(name="opool", bufs=2) as opool,
            tc.tile_pool(name="pspool", bufs=2, space="PSUM") as pspool,
        ):
            e_sb = wpool.tile([128, FTILES, EK], bf16)
            nc.sync.dma_start(
                out=e_sb[:], in_=emb[:].rearrange("p (t k) -> p t k", t=FTILES)
            )
            e2_sb = wpool.tile([128, FTILES, EK], bf16)
            nc.vector.tensor_mul(e2_sb[:], e_sb[:], e_sb[:])

            for c in range(nch):
                ps = pspool.tile([128, w], f32, space="PSUM")
                for o in range(NOCT):
                    x_sb = xpool.tile([128, OCT, w], bf16)
                    off = (c * NOCT + o) * OCT * w
                    nc.sync.dma_start(
                        out=x_sb[:],
                        in_=xt[:, off : off + OCT * w].rearrange(
                            "p (h j) -> p h j", h=OCT
                        ),
                    )
                    xq_sb = qpool.tile([128, OCT, w], bf16)
                    nc.vector.tensor_mul(xq_sb[:], x_sb[:], x_sb[:])
                    for h in range(OCT):
                        t = o * OCT + h
                        gL = 32 * (t & 1)
                        gR = 64 + 32 * (t & 1)
                        first = t < 2
                        last = t >= FTILES - 2
                        nc.tensor.matmul(
                            ps[gL : gL + 32, :],
                            e_sb[:, t, :],
                            x_sb[:, h, :],
                            start=first,
                            stop=last,
                            tile_position=(0, gL),
                        )
                        nc.tensor.matmul(
                            ps[gR : gR + 32, :],
                            e2_sb[:, t, :],
                            xq_sb[:, h, :],
                            start=first,
                            stop=last,
                            tile_position=(0, gR),
                        )
                # L = g0 + g1, R = g2 + g3; out = L*L - R (0.5 folded into E')
                lt = opool.tile([EK, w], f32, name="lt")
                nc.vector.tensor_copy(lt[:], ps[0:32, :])
                nc.vector.tensor_add(lt[:], lt[:], ps[32:64, :])
                rt = opool.tile([EK, w], f32, name="rt")
                nc.vector.tensor_copy(rt[:], ps[64:96, :])
                nc.vector.tensor_add(rt[:], rt[:], ps[96:128, :])
                osb = opool.tile([EK, w], f32, name="osb")
                nc.vector.tensor_mul(osb[:], lt[:], lt[:])
                nc.vector.tensor_sub(osb[:], osb[:], rt[:])
                nc.sync.dma_start(out=outT[:, c * w : (c + 1) * w], in_=osb[:])

    nc.compile()
    return nc


def _get_program(nch: int, w: int):
    key = (nch, w)
    if key not in _PROGRAM_CACHE:
        _PROGRAM_CACHE[key] = _build_program(nch, w)
    return _PROGRAM_CACHE[key]


def _np_bf16():
    import concourse.mybir as mybir

    return mybir.dt.np(mybir.dt.bfloat16)


def _prepare_in_maps(input, emb_weight, train_idx):
    x = np.asarray(input, dtype=np.float32)
    e = np.asarray(emb_weight, dtype=np.float32)
    idx = np.asarray(train_idx).astype(np.int64)
    bf16 = _np_bf16()

    rows = np.unique(idx)
    U = len(rows)
    P0 = -(-U // CORES)
    nch = max(1, -(-P0 // MAXW))
    w = -(-P0 // nch)
    P = nch * w
    # pad the row list with repeats of the last row (recomputed harmlessly)
    rows_pad = np.concatenate([rows, np.full(CORES * P - U, rows[-1], np.int64)])
    core_rows = rows_pad.reshape(CORES, P)

    ep = np.zeros((F_PAD, EK), dtype=np.float32)
    ep[:F] = e * np.float32(1.0 / math.sqrt(2.0))
    emb_bf = np.ascontiguousarray(
        ep.reshape(FTILES, FP, EK).transpose(1, 0, 2)
    ).reshape(128, FTILES * EK).astype(bf16)

    in_maps = []
    for c in range(CORES):
        xp = np.zeros((P, F_PAD), dtype=bf16)
        xp[:, :F] = x[core_rows[c]].astype(bf16)
        # [P, F_PAD] -> [p, c, t, j] so per-partition flat order is (c, t, j)
        a = xp.reshape(nch, w, FTILES, FP).transpose(3, 0, 2, 1)
        xt_host = np.ascontiguousarray(a).reshape(128, FTILES * P)
        in_maps.append({"xt": xt_host, "emb": emb_bf})
    return in_maps, (nch, w), core_rows


def run_sharded(input, emb_weight, train_idx, trace: bool = False):
    """Run on 8 cores; returns (full_output, BassKernelResults)."""
    from concourse.bass_utils import run_bass_kernel_spmd

    in_maps, (nch, w), core_rows = _prepare_in_maps(input, emb_weight, train_idx)
    nc = _get_program(nch, w)
    res = run_bass_kernel_spmd(
        nc, in_maps, core_ids=list(range(CORES)), trace=trace
    )
    out = np.zeros((N_ROWS, EK), dtype=np.float32)
    for c in range(CORES):
        out[core_rows[c]] = res.results[c]["outT"].T
    return out, res


def kernel(input, emb_weight, train_idx):
    out, _ = run_sharded(input, emb_weight, train_idx)
    return out


# revision 8
# speedup vs baseline: 7.7745x; 1.0798x over previous
"""FM bi-interaction (embedding_lookup) Trainium2 kernel.

out[n, k] = 0.5 * ((x @ E)^2 - (x*x) @ (E*E))[n, k] * mask[n]
mask[n] = 1 if n in train_idx else 0

Strategy (data-parallel over rows, 8 NeuronCores):
- Only rows present in train_idx have nonzero output (~11k of 20k). The host
  gathers the unique train rows, splits them evenly across the 8 cores, and
  scatters the per-row results back into a zero output — no on-device mask.
- x is uploaded in bf16 (the 2e-2 rel-err gate leaves ~40x headroom), halving
  HBM traffic; E is pre-scaled by 1/sqrt(2) on the host so the 0.5 factor
  folds into the matmuls (out = L^2 - R with L = x@E', R = x^2@E'^2).
- Host packs x into the exact SBUF tile layout ([128 f-partitions, 8 f-tiles,
  w rows] per block, f padded 10000->10240), so every x DMA is one ~0.9 MB
  transfer with fully contiguous per-partition lines. DMAs alternate between
  the SP and ACT HWDGE rings to overlap.
- L matmuls (M=32) run as two accumulation streams (even/odd f-tiles) in
  distinct 32-column PE groups via tile_position, sharing one PSUM bank.
- R matmuls run in fp8: x^2 is squared into fp8e4 (alternating DVE/GpSimd),
  E'^2 is host-packed in fp8e4 scaled by 2^11 (dodging the fp8 subnormal
  floor; the epilogue multiplies by -2^-11). DoubleRow perf mode contracts
  two f-tiles per instruction at half the PE stream cost.
- The epilogue folds partition groups and computes L*L - R with 3 DVE ops.
"""

import math
import sys

if "/opt/trn_rl_repo" not in sys.path:
    sys.path.insert(0, "/opt/trn_rl_repo")

import numpy as np

N_ROWS = 20000
F = 10000
EK = 32
CORES = 8
FP = 128  # f-rows per tile (on SBUF partitions)
FTILES = 80
F_PAD = FP * FTILES  # 10240
OCT = 8  # f-tiles per DMA block
NOCT = FTILES // OCT  # 10
MAXW = 512  # PSUM bank limit (f32 columns)
E2_SHIFT = 11  # e'^2 upload scale: 2^11 keeps values out of fp8 subnormals

_PROGRAM_CACHE: dict = {}


def _build_program(nch: int, w: int):
    """Per-core Bass program: nch chunks of w rows each (w <= 512, w % 16 == 0)."""
    import concourse.mybir as mybir
    import concourse.tile as tile
    from concourse import bacc

    f32 = mybir.dt.float32
    bf16 = mybir.dt.bfloat16
    fp8 = mybir.dt.float8e4

    P = nch * w
    nc = bacc.Bacc("TRN2", target_bir_lowering=False, debug=False)
    # packed x: per partition p, flat index (c*FTILES + t)*w + j holds
    # x[row base_c + j, f = t*128 + p] (bf16, f >= 10000 zero-padded)
    xt = nc.dram_tensor("xt", [128, FTILES * P], bf16, kind="ExternalInput")
    # packed E/sqrt(2): per partition p, flat t*EK + k = E'[t*128 + p, k]
    emb = nc.dram_tensor("emb", [128, FTILES * EK], bf16, kind="ExternalInput")
    # packed (E/sqrt(2))^2 * 2^E2_SHIFT in fp8e4, f-tile PAIRS interleaved for
    # DoubleRow: flat (j*2 + i)*EK + k = E2'[(2j+i)*128 + p, k]
    emb2 = nc.dram_tensor("emb2", [128, FTILES * EK], fp8, kind="ExternalInput")
    outT = nc.dram_tensor("outT", [EK, P], f32, kind="ExternalOutput")

    with tile.TileContext(nc) as tc:
        with (
            tc.tile_pool(name="wpool", bufs=1) as wpool,
            tc.tile_pool(name="xpool", bufs=6) as xpool,
            tc.tile_pool(name="qpool", bufs=4) as qpool,
            tc.tile_pool(name="opool", bufs=2) as opool,
            tc.tile_pool(name="pspool", bufs=2, space="PSUM") as pspool,
        ):
            e_sb = wpool.tile([128, FTILES, EK], bf16)
            nc.sync.dma_start(
                out=e_sb[:], in_=emb[:].rearrange("p (t k) -> p t k", t=FTILES)
            )
            e2_sb = wpool.tile([128, FTILES // 2, 2, EK], fp8)
            nc.scalar.dma_start(
                out=e2_sb[:],
                in_=emb2[:].rearrange("p (j i k) -> p j i k", j=FTILES // 2, i=2),
            )

            for c in range(nch):
                # bank A: L accumulates over even/odd f-tiles in partition
                # groups 0-31/32-63; bank B: R (DoubleRow needs dst base 0)
                psbA = pspool.tile([128, 512], f32, space="PSUM", name="psA")
                psbB = pspool.tile([128, 512], f32, space="PSUM", name="psB")
                ps = psbA[:, :w]
                psR = psbB[0:32, :w]
                for o in range(NOCT):
                    x_sb = xpool.tile([128, OCT, w], bf16)
                    off = (c * NOCT + o) * OCT * w
                    dma_eng = nc.sync if o % 2 == 0 else nc.scalar
                    dma_eng.dma_start(
                        out=x_sb[:],
                        in_=xt[:, off : off + OCT * w].rearrange(
                            "p (h j) -> p h j", h=OCT
                        ),
                    )
                    xq_sb = qpool.tile([128, OCT, w], fp8)
                    # balance squares: Pool is cheaper per op, give it 17/30
                    g = c * NOCT + o
                    sq_eng = nc.vector if g % 7 < 3 else nc.gpsimd
                    sq_eng.tensor_mul(xq_sb[:], x_sb[:], x_sb[:])
                    for h in range(OCT):
                        t = o * OCT + h
                        gL = 32 * (t & 1)
                        nc.tensor.matmul(
                            ps[gL : gL + 32, :],
                            e_sb[:, t, :],
                            x_sb[:, h, :],
                            start=(t < 2),
                            stop=(t >= FTILES - 2),
                            tile_position=(0, gL),
                            skip_group_check=True,
                        )
                    for i in range(OCT // 2):
                        j = o * (OCT // 2) + i  # f-tile pair index
                        nc.tensor.matmul(
                            psR,
                            e2_sb[:, j, :, :],
                            xq_sb[:, 2 * i : 2 * i + 2, :],
                            start=(j == 0),
                            stop=(j == FTILES // 2 - 1),
                            skip_group_check=True,
                            perf_mode=mybir.MatmulPerfMode.DoubleRow,
                        )
                # out = L^2 - R*2^-E2_SHIFT, L = g0 + g1
                # (DVE may read at most ONE operand from PSUM per instruction)
                lt = opool.tile([EK, w], f32, name="lt")
                nc.vector.tensor_copy(lt[:], ps[0:32, :])
                nc.vector.tensor_add(lt[:], lt[:], ps[32:64, :])
                osb = opool.tile([EK, w], f32, name="osb")
                nc.vector.tensor_mul(osb[:], lt[:], lt[:])
                nc.vector.scalar_tensor_tensor(
                    out=osb[:],
                    in0=psR,
                    scalar=-(2.0 ** -E2_SHIFT),
                    in1=osb[:],
                    op0=mybir.AluOpType.mult,
                    op1=mybir.AluOpType.add,
                )
                nc.sync.dma_start(out=outT[:, c * w : (c + 1) * w], in_=osb[:])

    nc.compile()
    return nc


def _get_program(nch: int, w: int):
    key = (nch, w)
    if key not in _PROGRAM_CACHE:
        _PROGRAM_CACHE[key] = _build_program(nch, w)
    return _PROGRAM_CACHE[key]


def _np_dt(which: str):
    import concourse.mybir as mybir

    return mybir.dt.np(getattr(mybir.dt, which))


def _prepare_in_maps(input, emb_weight, train_idx):
    x = np.asarray(input, dtype=np.float32)
    e = np.asarray(emb_weight, dtype=np.float32)
    idx = np.asarray(train_idx).astype(np.int64)
    bf16 = _np_dt("bfloat16")
    fp8 = _np_dt("float8e4")

    rows = np.unique(idx)
    U = len(rows)
    P0 = -(-U // CORES)
    nch = max(1, -(-P0 // MAXW))
    w = -(-(-(-P0 // nch)) // 16) * 16  # ceil(P0/nch) rounded up to x16
    P = nch * w
    # pad the row list with repeats of the last row (recomputed harmlessly)
    rows_pad = np.concatenate([rows, np.full(CORES * P - U, rows[-1], np.int64)])
    core_rows = rows_pad.reshape(CORES, P)

    ep = np.zeros((F_PAD, EK), dtype=np.float32)
    ep[:F] = e * np.float32(1.0 / math.sqrt(2.0))
    emb_bf = np.ascontiguousarray(
        ep.reshape(FTILES, FP, EK).transpose(1, 0, 2)
    ).reshape(128, FTILES * EK).astype(bf16)
    e2 = (ep * ep) * np.float32(2.0 ** E2_SHIFT)
    emb2_f8 = np.ascontiguousarray(
        e2.reshape(FTILES, FP, EK).transpose(1, 0, 2)
    ).reshape(128, FTILES * EK).astype(fp8)

    in_maps = []
    for c in range(CORES):
        xp = np.zeros((P, F_PAD), dtype=bf16)
        xp[:, :F] = x[core_rows[c]].astype(bf16)
        # [P, F_PAD] -> [p, c, t, j] so per-partition flat order is (c, t, j)
        a = xp.reshape(nch, w, FTILES, FP).transpose(3, 0, 2, 1)
        xt_host = np.ascontiguousarray(a).reshape(128, FTILES * P)
        in_maps.append({"xt": xt_host, "emb": emb_bf, "emb2": emb2_f8})
    return in_maps, (nch, w), core_rows


def run_sharded(input, emb_weight, train_idx, trace: bool = False):
    """Run on 8 cores; returns (full_output, BassKernelResults)."""
    from concourse.bass_utils import run_bass_kernel_spmd

    in_maps, (nch, w), core_rows = _prepare_in_maps(input, emb_weight, train_idx)
    nc = _get_program(nch, w)
    res = run_bass_kernel_spmd(
        nc, in_maps, core_ids=list(range(CORES)), trace=trace
    )
    out = np.zeros((N_ROWS, EK), dtype=np.float32)
    for c in range(CORES):
        out[core_rows[c]] = res.results[c]["outT"].T
    return out, res


def kernel(input, emb_weight, train_idx):
    out, _ = run_sharded(input, emb_weight, train_idx)
    return out


# revision 26
# speedup vs baseline: 14.4432x; 1.8578x over previous
"""FM bi-interaction (embedding_lookup) Trainium2 kernel.

out[n, k] = 0.5 * ((x @ E)^2 - (x*x) @ (E*E))[n, k] * mask[n]
mask[n] = 1 if n in train_idx else 0

Strategy (data-parallel over rows, 8 NeuronCores):
- Only rows present in train_idx have nonzero output (~11k of 20k). The host
  gathers the unique train rows, splits them evenly across the 8 cores, and
  scatters the per-row results back into a zero output — no on-device mask.
- x is uploaded in bf16 (the 2e-2 rel-err gate leaves ~40x headroom), halving
  HBM traffic; E is pre-scaled by 1/sqrt(2) on the host so the 0.5 factor
  folds into the matmuls (out = L^2 - R with L = x@E', R = x^2@E'^2).
- Host packs x into the exact SBUF tile layout ([128 f-partitions, 16
  f-tiles, w rows] per block, f padded 10000->10240 — tiles must span all 128
  partitions or DMA throughput collapses), so every x DMA is one ~1.9 MB
  transfer with fully contiguous per-partition lines. DMAs alternate between
  the SP and ACT HWDGE rings to overlap.
- L matmuls (M=32) run as two accumulation streams (even/odd f-tiles) in
  distinct 32-column PE groups via tile_position, sharing one PSUM bank.
- R matmuls run in fp8: x^2 is squared into fp8e4 (DVE tensor_mul for 3/5
  blocks, ACT Square activation for 2/5 — GpSimd is far too slow on HW),
  E'^2 is host-packed in fp8e4 scaled by 2^11 (dodging the fp8 subnormal
  floor; the epilogue multiplies by -2^-11). DoubleRow perf mode contracts
  two f-tiles per instruction at half the PE stream cost.
- The epilogue folds partition groups and computes L*L - R with 3 DVE ops.
"""

import math
import sys

if "/opt/trn_rl_repo" not in sys.path:
    sys.path.insert(0, "/opt/trn_rl_repo")

import numpy as np

N_ROWS = 20000
F = 10000
EK = 32
CORES = 8
FP = 128  # f-rows per tile (on SBUF partitions; 125 partitions cripples HW DMA)
FTILES = 80
F_PAD = FP * FTILES  # 10240 (f padded with zeros)
OCT = 16  # f-tiles per DMA block (double-octet: ~1.9 MB DMAs, fewer DVE ops)
NOCT = FTILES // OCT  # 5
MAXW = 512  # PSUM bank limit (f32 columns)
E2_SHIFT = 11  # e'^2 upload scale: 2^11 keeps values out of fp8 subnormals

_PROGRAM_CACHE: dict = {}


def _build_program(nch: int, w: int, repeat: int = 1, hw_loop: int = 1):
    """Per-core Bass program: nch chunks of w rows each (w <= 512, w % 16 == 0).

    repeat > 1 re-runs the whole compute that many times inside the program
    (idempotent; test-only, for overhead-free device timing via the r-slope).
    hw_loop > 1 wraps the compute in a hardware For_i loop instead (test-only;
    multiplies device work without growing the instruction count).
    """
    import concourse.mybir as mybir
    import concourse.tile as tile
    from concourse import bacc

    f32 = mybir.dt.float32
    bf16 = mybir.dt.bfloat16
    fp8 = mybir.dt.float8e4

    P = nch * w
    nc = bacc.Bacc("TRN2", target_bir_lowering=False, debug=False)
    # packed x: per partition p, flat index (c*FTILES + t)*w + j holds
    # x[row base_c + j, f = t*128 + p] (bf16, f >= 10000 zero-padded; tiles
    # must span all 128 partitions — 125-partition DMA is ~2.6x slower)
    xt = nc.dram_tensor("xt", [FP, FTILES * P], bf16, kind="ExternalInput")
    # packed E/sqrt(2): per partition p, flat t*EK + k = E'[t*128 + p, k]
    # (f >= 10000 zero-padded)
    emb = nc.dram_tensor("emb", [FP, FTILES * EK], bf16, kind="ExternalInput")
    # packed (E/sqrt(2))^2 * 2^E2_SHIFT in fp8e4, f-tile PAIRS interleaved for
    # DoubleRow: flat (j*2 + i)*EK + k = E2'[(2j+i)*128 + p, k]
    emb2 = nc.dram_tensor("emb2", [FP, FTILES * EK], fp8, kind="ExternalInput")
    outT = nc.dram_tensor("outT", [EK, P], f32, kind="ExternalOutput")

    with tile.TileContext(nc) as tc:
        with (
            tc.tile_pool(name="wpool", bufs=1) as wpool,
            tc.tile_pool(name="xpool", bufs=5) as xpool,
            tc.tile_pool(name="qpool", bufs=4) as qpool,
            tc.tile_pool(name="opool", bufs=2) as opool,
            tc.tile_pool(name="pspool", bufs=2, space="PSUM") as pspool,
        ):
            e_sb = wpool.tile([FP, FTILES, EK], bf16)
            nc.sync.dma_start(
                out=e_sb[:], in_=emb[:].rearrange("p (t k) -> p t k", t=FTILES)
            )
            e2_sb = wpool.tile([FP, FTILES // 2, 2, EK], fp8)
            nc.scalar.dma_start(
                out=e2_sb[:],
                in_=emb2[:].rearrange("p (j i k) -> p j i k", j=FTILES // 2, i=2),
            )

            def emit_chunk(c):
                # bank A: L accumulates over even/odd f-tiles in partition
                # groups 0-31/32-63; bank B: R (DoubleRow needs dst base 0)
                psbA = pspool.tile([128, 512], f32, space="PSUM", name="psA")
                psbB = pspool.tile([128, 512], f32, space="PSUM", name="psB")
                ps = psbA[:, :w]
                psR = psbB[0:32, :w]
                for o in range(NOCT):
                    x_sb = xpool.tile([FP, OCT, w], bf16)
                    off = (c * NOCT + o) * OCT * w
                    dma_eng = nc.sync if o % 2 == 0 else nc.scalar
                    dma_eng.dma_start(
                        out=x_sb[:],
                        in_=xt[:, off : off + OCT * w].rearrange(
                            "p (h j) -> p h j", h=OCT
                        ),
                    )
                    xq_sb = qpool.tile([FP, OCT, w], fp8)
                    # squares: DVE for 3/5 blocks, ACT (Square activation)
                    # for 2/5 — GpSimd is far too slow on real HW
                    if o % 2 == 0:
                        nc.vector.tensor_mul(xq_sb[:], x_sb[:], x_sb[:])
                    else:
                        nc.scalar.activation(
                            out=xq_sb[:],
                            in_=x_sb[:],
                            func=mybir.ActivationFunctionType.Square,
                        )
                    for h in range(OCT):
                        t = o * OCT + h
                        gL = 32 * (t & 1)
                        nc.tensor.matmul(
                            ps[gL : gL + 32, :],
                            e_sb[:, t, :],
                            x_sb[:, h, :],
                            start=(t < 2),
                            stop=(t >= FTILES - 2),
                            tile_position=(0, gL),
                            skip_group_check=True,
                        )
                    for i in range(OCT // 2):
                        j = o * (OCT // 2) + i  # f-tile pair index
                        nc.tensor.matmul(
                            psR,
                            e2_sb[:, j, :, :],
                            xq_sb[:, 2 * i : 2 * i + 2, :],
                            start=(j == 0),
                            stop=(j == FTILES // 2 - 1),
                            skip_group_check=True,
                            perf_mode=mybir.MatmulPerfMode.DoubleRow,
                        )
                # out = L^2 - R*2^-E2_SHIFT, L = g0 + g1, on DVE (GPSIMD
                # cannot access PSUM and is slow; DVE reads at most one PSUM
                # operand per instruction)
                lt = opool.tile([EK, w], f32, name="lt")
                nc.vector.tensor_copy(lt[:], ps[0:32, :])
                nc.vector.tensor_add(lt[:], lt[:], ps[32:64, :])
                osb = opool.tile([EK, w], f32, name="osb")
                nc.vector.tensor_mul(osb[:], lt[:], lt[:])
                nc.vector.scalar_tensor_tensor(
                    out=osb[:],
                    in0=psR,
                    scalar=-(2.0 ** -E2_SHIFT),
                    in1=osb[:],
                    op0=mybir.AluOpType.mult,
                    op1=mybir.AluOpType.add,
                )
                nc.sync.dma_start(out=outT[:, c * w : (c + 1) * w], in_=osb[:])

            if hw_loop > 1:
                with tc.For_i(0, hw_loop):
                    for c in range(nch):
                        emit_chunk(c)
            else:
                for c in [c for _ in range(repeat) for c in range(nch)]:
                    emit_chunk(c)

    nc.compile()
    return nc


def _get_program(nch: int, w: int):
    key = (nch, w)
    if key not in _PROGRAM_CACHE:
        _PROGRAM_CACHE[key] = _build_program(nch, w)
    return _PROGRAM_CACHE[key]


def _np_dt(which: str):
    import concourse.mybir as mybir

    return mybir.dt.np(getattr(mybir.dt, which))


def _prepare_in_maps(input, emb_weight, train_idx):
    x = np.asarray(input, dtype=np.float32)
    e = np.asarray(emb_weight, dtype=np.float32)
    idx = np.asarray(train_idx).astype(np.int64)
    bf16 = _np_dt("bfloat16")
    fp8 = _np_dt("float8e4")

    rows = np.unique(idx)
    U = len(rows)
    if U == 0:
        return None, (0, 0), None  # no train rows: output is all zeros
    P0 = -(-U // CORES)
    nch = max(1, -(-P0 // MAXW))
    w = -(-(-(-P0 // nch)) // 16) * 16  # ceil(P0/nch) rounded up to x16
    P = nch * w
    # pad the row list with repeats of the last row (recomputed harmlessly)
    rows_pad = np.concatenate([rows, np.full(CORES * P - U, rows[-1], np.int64)])
    core_rows = rows_pad.reshape(CORES, P)

    ep = np.zeros((F_PAD, EK), dtype=np.float32)
    ep[:F] = e * np.float32(1.0 / math.sqrt(2.0))
    emb_bf = np.ascontiguousarray(
        ep.reshape(FTILES, FP, EK).transpose(1, 0, 2)
    ).reshape(FP, FTILES * EK).astype(bf16)
    e2 = (ep * ep) * np.float32(2.0 ** E2_SHIFT)
    emb2_f8 = np.ascontiguousarray(
        e2.reshape(FTILES, FP, EK).transpose(1, 0, 2)
    ).reshape(FP, FTILES * EK).astype(fp8)

    in_maps = []
    for c in range(CORES):
        xp = np.zeros((P, F_PAD), dtype=bf16)
        xp[:, :F] = x[core_rows[c]].astype(bf16)
        # [P, F_PAD] -> [p, c, t, j] so per-partition flat order is (c, t, j)
        a = xp.reshape(nch, w, FTILES, FP).transpose(3, 0, 2, 1)
        xt_host = np.ascontiguousarray(a).reshape(FP, FTILES * P)
        in_maps.append({"xt": xt_host, "emb": emb_bf, "emb2": emb2_f8})
    return in_maps, (nch, w), core_rows


def run_sharded(input, emb_weight, train_idx, trace: bool = False):
    """Run on 8 cores; returns (full_output, BassKernelResults)."""
    from concourse.bass_utils import run_bass_kernel_spmd

    in_maps, (nch, w), core_rows = _prepare_in_maps(input, emb_weight, train_idx)
    if in_maps is None:  # empty train_idx
        return np.zeros((N_ROWS, EK), dtype=np.float32), None
    nc = _get_program(nch, w)
    res = run_bass_kernel_spmd(
        nc, in_maps, core_ids=list(range(CORES)), trace=trace
    )
    out = np.zeros((N_ROWS, EK), dtype=np.float32)
    for c in range(CORES):
        out[core_rows[c]] = res.results[c]["outT"].T
    return out, res


def kernel(input, emb_weight, train_idx):
    out, _ = run_sharded(input, emb_weight, train_idx)
    return out
